# revision 12
# baseline (speedup 1.0000x reference)
"""GAT layer (N=8192, F_IN=256, H=64 per head, K=8 heads) on 8 Trainium2 cores.

Row-sharded, no collectives.  Per head k (reference):
    h   = features @ W[k]                      [N, H]
    wh1 = h @ a[k,:H]; wh2 = h @ a[k,H:]       [N]
    e   = leaky_relu(wh1[:,None] + wh2[None,:], 0.2)
    att = softmax(where(adj>0, e, -9e15), axis=1)
    out = elu(att @ h)

Algebra: with s = wh1_i + wh2_j,
    exp(lrelu(s)) = max(e^s, e^{0.2 s})
Divide by the row factor e^{0.2 wh1_i} (cancels in softmax):
    f_ij / e1_i = max(G1_i * E2_j, e2_j) = max(G1_i, inv2_j) * E2_j
with G1 = e^{0.8 wh1}, E2 = e^{wh2}, e2 = e^{0.2 wh2}, inv2 = e^{-0.8 wh2}.
The per-j factor E2_j folds into the value matrix (computed on host):
    unnorm[i,j] = adj_ij * max(G1_i, inv2_j)          <- ONE fused DVE/GPSIMD
                                                         scalar_tensor_tensor op
    num[i,:] | den[i] = sum_j unnorm[i,j] * [E2 h | E2]_j   <- PE matmul
    out[i]   = elu(num / den)

Device layout per core (row block r0 = core*1024, R = 1024 rows):
    adjr [128, 64, 1024] bf16 : adjr[p,g,i] = adj[r0+i, g*128+p]  (j on partitions)
    g1b  [128, 8, 1024] bf16  : G1 row, broadcast across partitions
    inv2 [128, 8, 64] f32     : inv2[p,k,g] = e^{-0.8 wh2}[g*128+p, k]
    hval [128, 64, 8, 65] bf16: hval[p,g,k,:] = [ (h*E2)[g*128+p, k-block] | E2 ]

Phase B: for each i-half (512 columns of out rows), 64 j-groups:
    u'_k [128, 512] = (g1b_k max inv2_k[p]) * adj      (STT, DVE or GPSIMD)
    acc_k,b [128 i, 65] += u'_k[:, b*128:(b+1)*128].T @ hval[g,k]   (PE)
PSUM accs are [128, 128]-padded (512 B) so 4 pack per 2 KB bank exactly:
32 accs = all 8 banks per half.  Drain: reciprocal of col 64, scale, elu,
DMA straight out (no transposes).
"""

import sys

sys.path.insert(0, "/opt/trn_rl_repo")

import numpy as np
import ml_dtypes
from contextlib import ExitStack

import concourse.bass as bass
import concourse.tile as tile
from concourse import bacc, mybir
from concourse.bass_utils import run_bass_kernel_spmd

N = 8192
F_IN = 256
H = 64
K = 8
N_CORES = 8
R = N // N_CORES          # 1024 rows per core
G = N // 128              # 64 j-groups of 128
G_SUB = 8                 # j-groups per adjacency DMA chunk
HA = H + 1                # 65: head value columns + denominator column
NH = 2                    # i-halves
IW = R // NH              # 512 out-rows per half
IB = IW // 128            # 4 i-blocks of 128 per half

F32 = mybir.dt.float32
BF16 = mybir.dt.bfloat16
AX = mybir.AluOpType
AF = mybir.ActivationFunctionType

_cached = {}


def build_program(loop_t=1, eng="vgvsvgsv"):
    """eng[k]: engine split for head k's score+mask ops.  Real neuronxcc
    rejects scalar_tensor_tensor on Pool, so GPSIMD only runs TS/TT.
    'v': z = max(g1b, inv2) on DVE TS (4x) + u = z*adj on DVE TT (2x)
    's': z on DVE TS (4x) + mask TT on GPSIMD
    'g': z on GPSIMD TS + mask TT on GPSIMD"""
    key = (loop_t, eng)
    if key in _cached:
        return _cached[key]

    nc = bacc.Bacc("TRN2", target_bir_lowering=False, debug=False,
                   num_devices=N_CORES)

    adjr_d = nc.dram_tensor("adjr", [128, G, R], BF16, kind="ExternalInput").ap()
    g1b_d = nc.dram_tensor("g1b", [128, K, R], BF16, kind="ExternalInput").ap()
    inv2_d = nc.dram_tensor("inv2", [128, K, G], F32, kind="ExternalInput").ap()
    hval_d = nc.dram_tensor("hval", [128, G, K, HA], BF16,
                            kind="ExternalInput").ap()
    out_d = nc.dram_tensor("out", [R, K * H], F32, kind="ExternalOutput").ap()

    with tile.TileContext(nc) as tc:
        with ExitStack() as ctx:
            const = ctx.enter_context(tc.tile_pool(name="const", bufs=1))
            adj_pool = ctx.enter_context(tc.tile_pool(name="adj", bufs=2))
            u_pool = ctx.enter_context(tc.tile_pool(name="u", bufs=6))
            stg_pool = ctx.enter_context(tc.tile_pool(name="stg", bufs=4))
            fin_pool = ctx.enter_context(tc.tile_pool(name="fin", bufs=8))
            psum = ctx.enter_context(tc.tile_pool(name="psum", bufs=8,
                                                  space="PSUM"))

            g1b_sb = const.tile([128, K, R], BF16)
            nc.sync.dma_start(g1b_sb[:], g1b_d[:])
            inv2_sb = const.tile([128, K, G], F32)
            nc.sync.dma_start(inv2_sb[:], inv2_d[:])
            hval_sb = const.tile([128, G, K, HA], BF16)
            nc.sync.dma_start(hval_sb[:], hval_d[:])

            loop_cm = tc.For_i(0, loop_t, 1) if loop_t > 1 else None
            if loop_cm is not None:
                ctx.enter_context(loop_cm)

            for half in range(NH):
                i0 = half * IW
                # 4 accumulators share one 2 KB PSUM bank: acc (k,b) lives at
                # free-offset slot*128 of bank tile (k*IB+b)//4.
                banks = [psum.tile([128, 4, 128], F32, tag="ps",
                                   name=f"bank{half}_{i}") for i in range(8)]
                accs = {}
                for k in range(K):
                    for b in range(IB):
                        idx = k * IB + b
                        accs[(k, b)] = banks[idx // 4][:, idx % 4, 0:HA]
                for gs in range(G // G_SUB):
                    adj_t = adj_pool.tile([128, G_SUB, IW], BF16)
                    nc.sync.dma_start(
                        adj_t[:],
                        adjr_d[:, gs * G_SUB:(gs + 1) * G_SUB, i0:i0 + IW],
                    )
                    for gi in range(G_SUB):
                        g = gs * G_SUB + gi
                        for k in range(K):
                            u = u_pool.tile([128, IW], BF16, tag="u")
                            z = u_pool.tile([128, IW], BF16, tag="z")
                            zeng = nc.gpsimd if eng[k] == "g" else nc.vector
                            meng = nc.vector if eng[k] == "v" else nc.gpsimd
                            zeng.tensor_scalar(
                                z[:], g1b_sb[:, k, i0:i0 + IW],
                                inv2_sb[:, k, g:g + 1], None, op0=AX.max)
                            meng.tensor_tensor(
                                u[:], z[:], adj_t[:, gi, :], op=AX.mult)
                            # 4 accs of head k share bank k; PE start zeroes
                            # the whole 2 KB zero-region, so only (g=0, b=0)
                            # starts and only (g=63, b=3) stops the group.
                            for b in range(IB):
                                nc.tensor.matmul(
                                    accs[(k, b)],
                                    u[:, b * 128:(b + 1) * 128],
                                    hval_sb[:, g, k, :],
                                    start=(g == 0 and b == 0),
                                    stop=(g == G - 1 and b == IB - 1),
                                )
                # drain this half: scale by 1/den, elu, store
                for b in range(IB):
                    stg = stg_pool.tile([128, K, HA], F32, tag="stg")
                    for k in range(K):
                        nc.scalar.copy(stg[:, k, :], accs[(k, b)])
                    recips = fin_pool.tile([128, K], F32, tag="recip")
                    nc.vector.reciprocal(recips[:], stg[:, :, H])
                    fin = fin_pool.tile([128, K, H], F32, tag="fin")
                    nc.gpsimd.tensor_tensor(
                        fin[:], stg[:, :, 0:H],
                        recips[:].unsqueeze(2).broadcast_to((128, K, H)),
                        op=AX.mult,
                    )
                    finf = fin[:].rearrange("p k f -> p (k f)")
                    # elu(x) = exp(min(x,0)) + (max(x,0) - 1)
                    fmin = fin_pool.tile([128, K * H], F32, tag="fmin")
                    nc.vector.tensor_scalar(fmin[:], finf, 0.0, None, op0=AX.min)
                    ex = fin_pool.tile([128, K * H], F32, tag="ex")
                    nc.scalar.activation(ex[:], fmin[:], AF.Exp)
                    rel = fin_pool.tile([128, K * H], F32, tag="rel")
                    nc.vector.tensor_scalar(rel[:], finf, 0.0, -1.0,
                                            op0=AX.max, op1=AX.add)
                    res = fin_pool.tile([128, K * H], F32, tag="res")
                    nc.gpsimd.tensor_tensor(res[:], ex[:], rel[:], op=AX.add)
                    nc.sync.dma_start(
                        out_d[i0 + b * 128:i0 + (b + 1) * 128, :], res[:])

    nc.compile()
    _cached[key] = nc
    return nc


def prepare_inputs(features, adj, W, a):
    """Host-side prep: projections h/wh1/wh2 + per-core sharded layouts."""
    features = np.asarray(features, dtype=np.float32)
    adj = np.asarray(adj, dtype=np.float32)
    W = np.asarray(W, dtype=np.float32)
    a = np.asarray(a, dtype=np.float32)

    # h[k] = features @ W[k]  -> [N, K, H]
    h = np.einsum("nf,kfh->nkh", features, W, optimize=True)
    wh1 = np.einsum("nkh,kh->nk", h, a[:, :H])           # [N, K]
    wh2 = np.einsum("nkh,kh->nk", h, a[:, H:])           # [N, K]
    G1 = np.exp(0.8 * wh1).astype(np.float32)
    INV2 = np.exp(-0.8 * wh2).astype(np.float32)
    E2 = np.exp(wh2).astype(np.float32)

    # hval[p, g, k, 0:H] = (h * E2)[g*128+p, k, :]; hval[..., H] = E2
    hval = np.empty((N, K, HA), dtype=np.float32)
    hval[:, :, 0:H] = h * E2[:, :, None]
    hval[:, :, H] = E2
    hval = np.ascontiguousarray(
        hval.reshape(G, 128, K, HA)).astype(ml_dtypes.bfloat16)
    hval = np.ascontiguousarray(hval.transpose(1, 0, 2, 3))  # [128, G, K, HA]

    # inv2[p, k, g] = INV2[g*128+p, k]
    inv2 = np.ascontiguousarray(
        INV2.reshape(G, 128, K).transpose(1, 2, 0))          # [128, K, G]

    in_maps = []
    for c in range(N_CORES):
        r0 = c * R
        # adjr[p, g, i] = adj[r0 + i, g*128 + p]
        blk = adj[r0:r0 + R, :]                              # [R, N]
        adj_r = np.ascontiguousarray(
            blk.reshape(R, G, 128).transpose(2, 1, 0)
        ).astype(ml_dtypes.bfloat16)                         # [128, G, R]
        # g1b[p, k, i] = G1[r0 + i, k]  broadcast across partitions
        g1_blk = G1[r0:r0 + R, :].T.astype(ml_dtypes.bfloat16)   # [K, R]
        g1b = np.ascontiguousarray(
            np.broadcast_to(g1_blk[None], (128, K, R)))
        in_maps.append({
            "adjr": adj_r,
            "g1b": g1b,
            "inv2": inv2,
            "hval": hval,
        })
    return in_maps


def kernel(features, adj, W, a):
    nc = build_program()
    in_maps = prepare_inputs(features, adj, W, a)
    res = run_bass_kernel_spmd(nc, in_maps, list(range(N_CORES)))
    out = np.concatenate(
        [res.results[c]["out"] for c in range(N_CORES)], axis=0)
    return out.astype(np.float32)


if __name__ == "__main__":
    rng = np.random.default_rng(0)
    features = rng.standard_normal((N, F_IN), dtype=np.float32)
    adj = (rng.integers(0, 2, size=(N, N))).astype(np.float32)
    W = (rng.standard_normal((K, F_IN, H), dtype=np.float32) * 0.118)
    a = (rng.standard_normal((K, 2 * H), dtype=np.float32) * 0.176)
    out = kernel(features=features, adj=adj, W=W, a=a)
    print("out", out.shape, out.dtype, np.abs(out).max())


# revision 18
# speedup vs baseline: 3.0666x; 3.0666x over previous
"""GAT layer (N=8192, F_IN=256, H=64 per head, K=8 heads) on 8 Trainium2 cores.

Strategy (row-sharding, fully data-parallel, no collectives):
  reference per head k:
    h   = features @ W[k]                      [N, H]
    wh1 = h @ a[k,:H]; wh2 = h @ a[k,H:]       [N]
    e   = leaky_relu(wh1[:,None] + wh2[None,:], 0.2)
    att = softmax(where(adj>0, e, -9e15), axis=1)
    out = elu(att @ h)

  Algebra: with s = wh1[i] + wh2[j],
    exp(lrelu(s)) = exp(0.2 s) * max(exp(0.8 s), 1)
                  = e1_{i} * e2_{j} * max(G1_i * G2_j, 1)
  where G1 = exp(0.8 wh1), G2 = exp(0.8 wh2), e1 = exp(0.2 wh1), e2 = exp(0.2 wh2).
  The row factor e1_i cancels in softmax.  The column factor e2_j is folded
  into the value matrix.  Masked entries: adj in {0,1} multiplies exactly.
    unnorm[i,j] = adj[i,j] * max(G1_i G2_j, 1) * e2_j   (up to the cancelled e1_i)
    out[i]      = elu( (unnorm @ h) / (unnorm @ 1) )
  On device the big [N/8, N] tensor per head is produced with only
  two DVE passes per tile (tensor_scalar mult+max fused, tensor_tensor mask)
  in bf16, and consumed by the PE with stationary [h*e2 | e2] per head.
  G1/G2/e2 come from tiny host matmuls (features @ (W[k] @ a[k])).

Per-core layout ([j,i]-transposed tiles so contraction j sits on partitions):
  adj_r  [128, 2, 64, 512] bf16 : adj_r[p,ib,g,i] = adj[r0+ib*512+i, g*128+p]
  g1b    [128, 8, 2, 512] bf16  : G1 row broadcast across partitions
  g2t    [128, 8, 64] f32       : g2t[p,k,g] = G2[g*128+p, k]  (per-partition scalars)
  e25t   [128, 64, 8] f32       : e25t[p,g,k] = e2[g*128+p, k]
  featT  [256, 8192] f32        : features.T (replicated; h computed on device)
  w_cat  [256, 512] f32         : all heads' W side by side
"""

import sys
import os

sys.path.insert(0, "/opt/trn_rl_repo")

import numpy as np
import ml_dtypes
from contextlib import ExitStack

import concourse.bass as bass
import concourse.tile as tile
from concourse import bacc, mybir
from concourse.bass_utils import run_bass_kernel_spmd

N = 8192
F_IN = 256
H = 64
K = 8
ALPHA = 0.2
N_CORES = 8
R = N // N_CORES          # 1024 rows per core
IB = 2                    # i-blocks per core (512 columns of out-rows each)
IW = R // IB              # 512, i-width per block
G = N // 128              # 64 j-groups of 128
G_SUB = 8                 # j-groups per adjacency DMA
HA = H + 1                # 65: head value columns + denominator column

F32 = mybir.dt.float32
BF16 = mybir.dt.bfloat16
AX = mybir.AluOpType

_cached = {}


def build_program(loop_t=1, z_eng="vvvvvaaa", m_eng="gvgvvvvv",
                  sweeps=((0, 1, 5, 6), (2, 3, 4, 7)), fold_e2=False):
    """z_eng[k]: engine producing head k's score tiles ('v'=DVE zm via
    tensor_scalar mult+max, 'a'=ACT v=relu(G2*g1b-1)=zm-1, 'g'=GPSIMD zm).
    m_eng[k]: engine for the mask multiply ('v' DVE / 'g' GPSIMD).
    ACT heads ('a') use [h*e2|e2] stationary and an extra correction
    matmul with rhs=adj; other heads fold e2 into the score via the
    tensor_scalar scalar2 AP: zm' = max(z*e2, e2).
    sweeps: two groups of 4 heads; each sweep holds 8 PSUM accumulators
    ([65,512] x 2 i-blocks) and re-reads the adjacency."""
    key = (loop_t, z_eng, m_eng, tuple(map(tuple, sweeps)), fold_e2)
    if key in _cached:
        return _cached[key]

    nc = bacc.Bacc("TRN2", target_bir_lowering=False, debug=False,
                   num_devices=N_CORES)

    adjr_d = nc.dram_tensor("adjr", [128, G, IB, IW], BF16, kind="ExternalInput").ap()
    g1b_d = nc.dram_tensor("g1b", [128, K, IB, IW], BF16, kind="ExternalInput").ap()
    g2t_d = nc.dram_tensor("g2t", [128, K, G], F32, kind="ExternalInput").ap()
    hval_d = nc.dram_tensor("hval", [128, G, K, HA], BF16,
                            kind="ExternalInput").ap()
    out_d = nc.dram_tensor("out", [R, K * H], F32, kind="ExternalOutput").ap()

    with tile.TileContext(nc) as tc:
        with ExitStack() as ctx:
            const = ctx.enter_context(tc.tile_pool(name="const", bufs=1))
            hpool = ctx.enter_context(tc.tile_pool(name="h", bufs=G))
            fpool = ctx.enter_context(tc.tile_pool(name="feat", bufs=4))
            e25x_pool = ctx.enter_context(tc.tile_pool(name="e25x", bufs=2))
            adj_pool = ctx.enter_context(tc.tile_pool(name="adj", bufs=2))
            z_pool = ctx.enter_context(tc.tile_pool(name="z", bufs=3))
            u_pool = ctx.enter_context(tc.tile_pool(name="u", bufs=3))
            stg_pool = ctx.enter_context(tc.tile_pool(name="stg", bufs=4))
            fin_pool = ctx.enter_context(tc.tile_pool(name="fin", bufs=2))
            acc_sb_pool = ctx.enter_context(tc.tile_pool(name="accsb", bufs=2))
            psum = ctx.enter_context(tc.tile_pool(name="psum", bufs=8, space="PSUM"))

            # ---- constants ----
            g1b_sb = const.tile([128, K, IB, IW], BF16)
            nc.sync.dma_start(g1b_sb[:], g1b_d[:])
            g2t_sb = const.tile([128, K, G], F32)
            nc.sync.dma_start(g2t_sb[:], g2t_d[:])
            # value matrix [h*e2 | e2] per (g, k): computed on host now
            hval_sb = const.tile([128, G, K, HA], BF16)
            nc.sync.dma_start(hval_sb[:], hval_d[:])
            ident = const.tile([128, 128], F32)
            from concourse.masks import make_identity
            make_identity(nc, ident[:])
            negone = const.tile([128, 1], F32)
            nc.vector.memset(negone[:], -1.0)

            loop_cm = tc.For_i(0, loop_t, 1) if loop_t > 1 else None
            if loop_cm is not None:
                ctx.enter_context(loop_cm)

            assert not fold_e2, "host hval provides [h*e2|e2] (act-type) only"
            act_heads = list(range(K))
            h_sb = [hval_sb[:, g, :, :] for g in range(G)]

            # ---- phase B: two 4-head sweeps over the adjacency ----
            assert fold_e2 or len(act_heads) == K
            PAIR = IB * IW  # 1024

            for sweep_heads in sweeps:
                accs = {}
                for k in sweep_heads:
                    for ib in range(IB):
                        accs[(k, ib)] = psum.tile(
                            [HA, IW], F32, tag="ps", name=f"acc{k}_{ib}")
                for gs in range(G // G_SUB):
                    adj_t = adj_pool.tile([128, G_SUB, IB, IW], BF16)
                    nc.sync.dma_start(
                        adj_t[:], adjr_d[:, gs * G_SUB:(gs + 1) * G_SUB, :, :]
                    )
                    for gi in range(G_SUB):
                        g = gs * G_SUB + gi
                        adj_pair = adj_t[:, gi, :, :].rearrange("p b i -> p (b i)")
                        z_all = z_pool.tile([128, 4, PAIR], BF16, tag="z")
                        for s, k in enumerate(sweep_heads):
                            g1b_pair = g1b_sb[:, k, :, :].rearrange(
                                "p b i -> p (b i)")
                            s1 = g2e_sb[:, k, g:g + 1] if fold_e2 else \
                                g2t_sb[:, k, g:g + 1]
                            s2 = e25t_sb[:, g, k:k + 1] if fold_e2 else 1.0
                            if z_eng[k] == "v":
                                # zm' = max(z*e2, e2) -- e2 folded in
                                nc.vector.tensor_scalar(
                                    z_all[:, s, :], g1b_pair, s1, s2,
                                    op0=AX.mult, op1=AX.max,
                                )
                            elif z_eng[k] == "g":
                                nc.gpsimd.tensor_scalar(
                                    z_all[:, s, :], g1b_pair, s1, s2,
                                    op0=AX.mult, op1=AX.max,
                                )
                            else:
                                # v = relu(G2*g1b - 1) = zm - 1 (unscaled)
                                nc.scalar.activation(
                                    z_all[:, s, :], g1b_pair,
                                    mybir.ActivationFunctionType.Relu,
                                    bias=negone[:],
                                    scale=g2t_sb[:, k, g:g + 1],
                                )
                        u_all = u_pool.tile([128, 4, PAIR], BF16, tag="u")
                        for s, k in enumerate(sweep_heads):
                            eng = nc.gpsimd if m_eng[k] == "g" else nc.vector
                            eng.tensor_tensor(
                                u_all[:, s, :], z_all[:, s, :], adj_pair,
                                op=AX.mult)
                        for s, k in enumerate(sweep_heads):
                            is_act = z_eng[k] == "a"
                            for ib in range(IB):
                                nc.tensor.matmul(
                                    accs[(k, ib)][:],
                                    h_sb[g][:, k, :],
                                    u_all[:, s, ib * IW:(ib + 1) * IW],
                                    start=(g == 0),
                                    stop=(g == G - 1 and not is_act),
                                )
                            if is_act:  # correction: acc += h_sb @ adj
                                for ib in range(IB):
                                    nc.tensor.matmul(
                                        accs[(k, ib)][:],
                                        h_sb[g][:, k, :],
                                        adj_t[:, gi, ib, :],
                                        start=False,
                                        stop=(g == G - 1),
                                    )
                # drain this sweep: transpose, divide, elu, store
                sw_sorted = sorted(sweep_heads)
                # contiguous head runs for the output DMA
                runs = []
                for k in sw_sorted:
                    if runs and runs[-1][-1] == k - 1:
                        runs[-1].append(k)
                    else:
                        runs.append([k])
                for ib in range(IB):
                    stgs = [stg_pool.tile([128, 4, HA], F32, tag="stg",
                                          name=f"stg{sweep_heads[0]}_{ib}_{c}")
                            for c in range(IW // 128)]
                    for s, k in enumerate(sw_sorted):
                        acc_sb = acc_sb_pool.tile([HA, IW], F32, tag="accsb")
                        nc.scalar.copy(acc_sb[:], accs[(k, ib)][:])
                        for c in range(IW // 128):
                            pst = psum.tile([128, HA], F32, tag="ps",
                                            name=f"pst{k}_{ib}_{c}")
                            nc.tensor.transpose(
                                pst[:], acc_sb[:, c * 128:(c + 1) * 128],
                                ident[0:HA, 0:HA],
                            )
                            nc.scalar.copy(stgs[c][:, s, :], pst[:])
                    for c in range(IW // 128):
                        stg = stgs[c]
                        recips = fin_pool.tile([128, 4], F32, tag="recip")
                        nc.vector.reciprocal(recips[:], stg[:, :, H])
                        fin = fin_pool.tile([128, 4, H], F32, tag="fin")
                        nc.vector.tensor_tensor(
                            fin[:], stg[:, :, 0:H],
                            recips[:].unsqueeze(2).broadcast_to((128, 4, H)),
                            op=AX.mult,
                        )
                        # elu(x) = exp(min(x,0)) + (max(x,0) - 1)
                        fin2 = fin_pool.tile([128, 4 * H], F32, tag="fin2")
                        finf = fin[:].rearrange("p k f -> p (k f)")
                        nc.vector.tensor_scalar(
                            fin2[:], finf, 0.0, None, op0=AX.min
                        )
                        ex = fin_pool.tile([128, 4 * H], F32, tag="ex")
                        nc.scalar.activation(
                            ex[:], fin2[:], mybir.ActivationFunctionType.Exp
                        )
                        rel = fin_pool.tile([128, 4 * H], F32, tag="rel")
                        nc.vector.tensor_scalar(
                            rel[:], finf, 0.0, -1.0, op0=AX.max, op1=AX.add
                        )
                        res = fin_pool.tile([128, 4 * H], F32, tag="res")
                        nc.vector.tensor_tensor(res[:], ex[:], rel[:], op=AX.add)
                        resv = res[:].rearrange("p (k f) -> p k f", k=4)
                        for run in runs:
                            s0 = sw_sorted.index(run[0])
                            nc.sync.dma_start(
                                out_d[ib * IW + c * 128:
                                      ib * IW + (c + 1) * 128,
                                      run[0] * H:(run[-1] + 1) * H],
                                resv[:, s0:s0 + len(run), :],
                            )

    nc.compile()
    _cached[key] = nc
    return nc


def prepare_inputs(features, adj, W, a):
    """Host-side prep: tiny projections + per-core sharded/transposed layouts."""
    features = np.asarray(features, dtype=np.float32)
    adj = np.asarray(adj, dtype=np.float32)
    W = np.asarray(W, dtype=np.float32)
    a = np.asarray(a, dtype=np.float32)

    # av[k] = W[k] @ a[k]  -> wh = features @ av.T   (tiny: K*F_IN*H flops)
    av1 = np.einsum("kfh,kh->kf", W, a[:, :H])          # [K, F_IN]
    av2 = np.einsum("kfh,kh->kf", W, a[:, H:])          # [K, F_IN]
    wh1 = features @ av1.T                               # [N, K]
    wh2 = features @ av2.T                               # [N, K]
    G1 = np.exp(0.8 * wh1).astype(np.float32)            # row factors
    G2 = np.exp(0.8 * wh2).astype(np.float32)            # col factors
    E25 = np.exp(0.2 * wh2).astype(np.float32)           # folded into values

    # g2t[p,k,g] = G2[g*128+p, k]
    g2t = np.ascontiguousarray(
        G2.reshape(G, 128, K).transpose(1, 2, 0))        # [128, K, G]

    # host value matrix: hval[p, g, k, :] = [h*e2 | e2] at node g*128+p
    h = np.einsum("nf,kfh->nkh", features, W, optimize=True)  # [N, K, H]
    hval = np.empty((N, K, HA), dtype=np.float32)
    hval[:, :, 0:H] = h * E25[:, :, None]
    hval[:, :, H] = E25
    hval = np.ascontiguousarray(
        hval.reshape(G, 128, K, HA).transpose(1, 0, 2, 3)
    ).astype(ml_dtypes.bfloat16)                         # [128, G, K, HA]

    in_maps = []
    for c in range(N_CORES):
        r0 = c * R
        # adj_r[p, g, ib, i] = adj[r0 + ib*IW + i, g*128 + p]
        blk = adj[r0:r0 + R, :]                          # [R, N]
        adj_r = np.ascontiguousarray(
            blk.reshape(IB, IW, G, 128).transpose(3, 2, 0, 1)
        ).astype(ml_dtypes.bfloat16)                     # [128, G, IB, IW]
        # g1b[p, k, ib, i] = G1[r0 + ib*IW + i, k]
        g1_blk = G1[r0:r0 + R, :].reshape(IB, IW, K).transpose(2, 0, 1)
        g1b = np.broadcast_to(
            g1_blk[None].astype(ml_dtypes.bfloat16), (128, K, IB, IW))
        g1b = np.ascontiguousarray(g1b)
        in_maps.append({
            "adjr": adj_r,
            "g1b": g1b,
            "g2t": g2t,
            "hval": hval,
        })
    return in_maps


def kernel(features, adj, W, a):
    nc = build_program()
    in_maps = prepare_inputs(features, adj, W, a)
    res = run_bass_kernel_spmd(nc, in_maps, list(range(N_CORES)))
    out = np.concatenate(
        [res.results[c]["out"] for c in range(N_CORES)], axis=0)
    return out.astype(np.float32)


if __name__ == "__main__":
    rng = np.random.default_rng(0)
    features = rng.standard_normal((N, F_IN), dtype=np.float32)
    adj = (rng.integers(0, 2, size=(N, N))).astype(np.float32)
    W = (rng.standard_normal((K, F_IN, H), dtype=np.float32) * 0.118)
    a = (rng.standard_normal((K, 2 * H), dtype=np.float32) * 0.176)
    out = kernel(features=features, adj=adj, W=W, a=a)
    print("out", out.shape, out.dtype, np.abs(out).max())



# revision 19
# speedup vs baseline: 4.0627x; 1.3248x over previous
"""GAT layer (N=8192, F_IN=256, H=64 per head, K=8 heads) on 8 Trainium2 cores.

Strategy (row-sharding, fully data-parallel, no collectives):
  reference per head k:
    h   = features @ W[k]                      [N, H]
    wh1 = h @ a[k,:H]; wh2 = h @ a[k,H:]       [N]
    e   = leaky_relu(wh1[:,None] + wh2[None,:], 0.2)
    att = softmax(where(adj>0, e, -9e15), axis=1)
    out = elu(att @ h)

  Algebra: with s = wh1[i] + wh2[j],
    exp(lrelu(s)) = exp(0.2 s) * max(exp(0.8 s), 1)
                  = e1_{i} * e2_{j} * max(G1_i * G2_j, 1)
  where G1 = exp(0.8 wh1), G2 = exp(0.8 wh2), e1 = exp(0.2 wh1), e2 = exp(0.2 wh2).
  The row factor e1_i cancels in softmax.  The column factor e2_j is folded
  into the value matrix.  Masked entries: adj in {0,1} multiplies exactly.
    unnorm[i,j] = adj[i,j] * max(G1_i G2_j, 1) * e2_j   (up to the cancelled e1_i)
    out[i]      = elu( (unnorm @ h) / (unnorm @ 1) )
  On device the big [N/8, N] tensor per head is produced with only
  two DVE passes per tile (tensor_scalar mult+max fused, tensor_tensor mask)
  in bf16, and consumed by the PE with stationary [h*e2 | e2] per head.
  G1/G2/e2 come from tiny host matmuls (features @ (W[k] @ a[k])).

Per-core layout ([j,i]-transposed tiles so contraction j sits on partitions):
  adj_r  [128, 2, 64, 512] bf16 : adj_r[p,ib,g,i] = adj[r0+ib*512+i, g*128+p]
  g1b    [128, 8, 2, 512] bf16  : G1 row broadcast across partitions
  g2t    [128, 8, 64] f32       : g2t[p,k,g] = G2[g*128+p, k]  (per-partition scalars)
  e25t   [128, 64, 8] f32       : e25t[p,g,k] = e2[g*128+p, k]
  featT  [256, 8192] f32        : features.T (replicated; h computed on device)
  w_cat  [256, 512] f32         : all heads' W side by side
"""

import sys
import os

sys.path.insert(0, "/opt/trn_rl_repo")

import numpy as np
import ml_dtypes
from contextlib import ExitStack

import concourse.bass as bass
import concourse.tile as tile
from concourse import bacc, mybir
from concourse.bass_utils import run_bass_kernel_spmd

N = 8192
F_IN = 256
H = 64
K = 8
ALPHA = 0.2
N_CORES = 8
R = N // N_CORES          # 1024 rows per core
IB = 2                    # i-blocks per core (512 columns of out-rows each)
IW = R // IB              # 512, i-width per block
G = N // 128              # 64 j-groups of 128
G_SUB = 8                 # j-groups per adjacency DMA
HA = H + 1                # 65: head value columns + denominator column

F32 = mybir.dt.float32
BF16 = mybir.dt.bfloat16
AX = mybir.AluOpType

_cached = {}


def build_program(loop_t=1, z_eng="vvvvvaaa", m_eng="vvvvvvvv",
                  sweeps=((0, 1, 5, 6), (2, 3, 4, 7)), fold_e2=False):
    """z_eng[k]: engine producing head k's score tiles ('v'=DVE zm via
    tensor_scalar mult+max, 'a'=ACT v=relu(G2*g1b-1)=zm-1, 'g'=GPSIMD zm).
    m_eng[k]: engine for the mask multiply ('v' DVE / 'g' GPSIMD).
    ACT heads ('a') use [h*e2|e2] stationary and an extra correction
    matmul with rhs=adj; other heads fold e2 into the score via the
    tensor_scalar scalar2 AP: zm' = max(z*e2, e2).
    sweeps: two groups of 4 heads; each sweep holds 8 PSUM accumulators
    ([65,512] x 2 i-blocks) and re-reads the adjacency."""
    key = (loop_t, z_eng, m_eng, tuple(map(tuple, sweeps)), fold_e2)
    if key in _cached:
        return _cached[key]

    nc = bacc.Bacc("TRN2", target_bir_lowering=False, debug=False,
                   num_devices=N_CORES)

    adjr_d = nc.dram_tensor("adjr", [128, G, IB, IW], BF16, kind="ExternalInput").ap()
    g1b_d = nc.dram_tensor("g1b", [128, K, IB, IW], BF16, kind="ExternalInput").ap()
    g2t_d = nc.dram_tensor("g2t", [128, K, G], F32, kind="ExternalInput").ap()
    hval_d = nc.dram_tensor("hval", [128, G, K, HA], BF16,
                            kind="ExternalInput").ap()
    out_d = nc.dram_tensor("out", [R, K * H], F32, kind="ExternalOutput").ap()

    with tile.TileContext(nc) as tc:
        with ExitStack() as ctx:
            const = ctx.enter_context(tc.tile_pool(name="const", bufs=1))
            hpool = ctx.enter_context(tc.tile_pool(name="h", bufs=G))
            fpool = ctx.enter_context(tc.tile_pool(name="feat", bufs=4))
            e25x_pool = ctx.enter_context(tc.tile_pool(name="e25x", bufs=2))
            adj_pool = ctx.enter_context(tc.tile_pool(name="adj", bufs=2))
            z_pool = ctx.enter_context(tc.tile_pool(name="z", bufs=3))
            u_pool = ctx.enter_context(tc.tile_pool(name="u", bufs=3))
            stg_pool = ctx.enter_context(tc.tile_pool(name="stg", bufs=4))
            fin_pool = ctx.enter_context(tc.tile_pool(name="fin", bufs=2))
            acc_sb_pool = ctx.enter_context(tc.tile_pool(name="accsb", bufs=2))
            psum = ctx.enter_context(tc.tile_pool(name="psum", bufs=8, space="PSUM"))

            # ---- constants ----
            g1b_sb = const.tile([128, K, IB, IW], BF16)
            nc.sync.dma_start(g1b_sb[:], g1b_d[:])
            g2t_sb = const.tile([128, K, G], F32)
            nc.sync.dma_start(g2t_sb[:], g2t_d[:])
            # value matrix [h*e2 | e2] per (g, k): computed on host now
            hval_sb = const.tile([128, G, K, HA], BF16)
            nc.sync.dma_start(hval_sb[:], hval_d[:])
            ident = const.tile([128, 128], F32)
            from concourse.masks import make_identity
            make_identity(nc, ident[:])
            negone = const.tile([128, 1], F32)
            nc.vector.memset(negone[:], -1.0)

            loop_cm = tc.For_i(0, loop_t, 1) if loop_t > 1 else None
            if loop_cm is not None:
                ctx.enter_context(loop_cm)

            assert not fold_e2, "host hval provides [h*e2|e2] (act-type) only"
            act_heads = list(range(K))
            h_sb = [hval_sb[:, g, :, :] for g in range(G)]

            # ---- phase B: two 4-head sweeps over the adjacency ----
            assert fold_e2 or len(act_heads) == K
            PAIR = IB * IW  # 1024

            for sweep_heads in sweeps:
                accs = {}
                for k in sweep_heads:
                    for ib in range(IB):
                        accs[(k, ib)] = psum.tile(
                            [HA, IW], F32, tag="ps", name=f"acc{k}_{ib}")
                for gs in range(G // G_SUB):
                    adj_t = adj_pool.tile([128, G_SUB, IB, IW], BF16)
                    nc.sync.dma_start(
                        adj_t[:], adjr_d[:, gs * G_SUB:(gs + 1) * G_SUB, :, :]
                    )
                    for gi in range(G_SUB):
                        g = gs * G_SUB + gi
                        adj_pair = adj_t[:, gi, :, :].rearrange("p b i -> p (b i)")
                        z_all = z_pool.tile([128, 4, PAIR], BF16, tag="z")
                        for s, k in enumerate(sweep_heads):
                            g1b_pair = g1b_sb[:, k, :, :].rearrange(
                                "p b i -> p (b i)")
                            s1 = g2e_sb[:, k, g:g + 1] if fold_e2 else \
                                g2t_sb[:, k, g:g + 1]
                            s2 = e25t_sb[:, g, k:k + 1] if fold_e2 else 1.0
                            if z_eng[k] == "v":
                                # zm' = max(z*e2, e2) -- e2 folded in
                                nc.vector.tensor_scalar(
                                    z_all[:, s, :], g1b_pair, s1, s2,
                                    op0=AX.mult, op1=AX.max,
                                )
                            elif z_eng[k] == "g":
                                nc.gpsimd.tensor_scalar(
                                    z_all[:, s, :], g1b_pair, s1, s2,
                                    op0=AX.mult, op1=AX.max,
                                )
                            else:
                                # v = relu(G2*g1b - 1) = zm - 1 (unscaled)
                                nc.scalar.activation(
                                    z_all[:, s, :], g1b_pair,
                                    mybir.ActivationFunctionType.Relu,
                                    bias=negone[:],
                                    scale=g2t_sb[:, k, g:g + 1],
                                )
                        u_all = u_pool.tile([128, 4, PAIR], BF16, tag="u")
                        for s, k in enumerate(sweep_heads):
                            eng = nc.gpsimd if m_eng[k] == "g" else nc.vector
                            eng.tensor_tensor(
                                u_all[:, s, :], z_all[:, s, :], adj_pair,
                                op=AX.mult)
                        for s, k in enumerate(sweep_heads):
                            is_act = z_eng[k] == "a"
                            for ib in range(IB):
                                nc.tensor.matmul(
                                    accs[(k, ib)][:],
                                    h_sb[g][:, k, :],
                                    u_all[:, s, ib * IW:(ib + 1) * IW],
                                    start=(g == 0),
                                    stop=(g == G - 1 and not is_act),
                                )
                            if is_act:  # correction: acc += h_sb @ adj
                                for ib in range(IB):
                                    nc.tensor.matmul(
                                        accs[(k, ib)][:],
                                        h_sb[g][:, k, :],
                                        adj_t[:, gi, ib, :],
                                        start=False,
                                        stop=(g == G - 1),
                                    )
                # drain this sweep: transpose, divide, elu, store
                sw_sorted = sorted(sweep_heads)
                # contiguous head runs for the output DMA
                runs = []
                for k in sw_sorted:
                    if runs and runs[-1][-1] == k - 1:
                        runs[-1].append(k)
                    else:
                        runs.append([k])
                for ib in range(IB):
                    stgs = [stg_pool.tile([128, 4, HA], F32, tag="stg",
                                          name=f"stg{sweep_heads[0]}_{ib}_{c}")
                            for c in range(IW // 128)]
                    for s, k in enumerate(sw_sorted):
                        acc_sb = acc_sb_pool.tile([HA, IW], F32, tag="accsb")
                        nc.scalar.copy(acc_sb[:], accs[(k, ib)][:])
                        for c in range(IW // 128):
                            pst = psum.tile([128, HA], F32, tag="ps",
                                            name=f"pst{k}_{ib}_{c}")
                            nc.tensor.transpose(
                                pst[:], acc_sb[:, c * 128:(c + 1) * 128],
                                ident[0:HA, 0:HA],
                            )
                            nc.scalar.copy(stgs[c][:, s, :], pst[:])
                    for c in range(IW // 128):
                        stg = stgs[c]
                        recips = fin_pool.tile([128, 4], F32, tag="recip")
                        nc.vector.reciprocal(recips[:], stg[:, :, H])
                        fin = fin_pool.tile([128, 4, H], F32, tag="fin")
                        nc.vector.tensor_tensor(
                            fin[:], stg[:, :, 0:H],
                            recips[:].unsqueeze(2).broadcast_to((128, 4, H)),
                            op=AX.mult,
                        )
                        # elu(x) = exp(min(x,0)) + (max(x,0) - 1)
                        fin2 = fin_pool.tile([128, 4 * H], F32, tag="fin2")
                        finf = fin[:].rearrange("p k f -> p (k f)")
                        nc.vector.tensor_scalar(
                            fin2[:], finf, 0.0, None, op0=AX.min
                        )
                        ex = fin_pool.tile([128, 4 * H], F32, tag="ex")
                        nc.scalar.activation(
                            ex[:], fin2[:], mybir.ActivationFunctionType.Exp
                        )
                        rel = fin_pool.tile([128, 4 * H], F32, tag="rel")
                        nc.vector.tensor_scalar(
                            rel[:], finf, 0.0, -1.0, op0=AX.max, op1=AX.add
                        )
                        res = fin_pool.tile([128, 4 * H], F32, tag="res")
                        nc.vector.tensor_tensor(res[:], ex[:], rel[:], op=AX.add)
                        resv = res[:].rearrange("p (k f) -> p k f", k=4)
                        for run in runs:
                            s0 = sw_sorted.index(run[0])
                            nc.sync.dma_start(
                                out_d[ib * IW + c * 128:
                                      ib * IW + (c + 1) * 128,
                                      run[0] * H:(run[-1] + 1) * H],
                                resv[:, s0:s0 + len(run), :],
                            )

    nc.compile()
    _cached[key] = nc
    return nc


def prepare_inputs(features, adj, W, a):
    """Host-side prep: tiny projections + per-core sharded/transposed layouts."""
    features = np.asarray(features, dtype=np.float32)
    adj = np.asarray(adj, dtype=np.float32)
    W = np.asarray(W, dtype=np.float32)
    a = np.asarray(a, dtype=np.float32)

    # av[k] = W[k] @ a[k]  -> wh = features @ av.T   (tiny: K*F_IN*H flops)
    av1 = np.einsum("kfh,kh->kf", W, a[:, :H])          # [K, F_IN]
    av2 = np.einsum("kfh,kh->kf", W, a[:, H:])          # [K, F_IN]
    wh1 = features @ av1.T                               # [N, K]
    wh2 = features @ av2.T                               # [N, K]
    G1 = np.exp(0.8 * wh1).astype(np.float32)            # row factors
    G2 = np.exp(0.8 * wh2).astype(np.float32)            # col factors
    E25 = np.exp(0.2 * wh2).astype(np.float32)           # folded into values

    # g2t[p,k,g] = G2[g*128+p, k]
    g2t = np.ascontiguousarray(
        G2.reshape(G, 128, K).transpose(1, 2, 0))        # [128, K, G]

    # host value matrix: hval[p, g, k, :] = [h*e2 | e2] at node g*128+p
    h = np.einsum("nf,kfh->nkh", features, W, optimize=True)  # [N, K, H]
    hval = np.empty((N, K, HA), dtype=np.float32)
    hval[:, :, 0:H] = h * E25[:, :, None]
    hval[:, :, H] = E25
    hval = np.ascontiguousarray(
        hval.reshape(G, 128, K, HA).transpose(1, 0, 2, 3)
    ).astype(ml_dtypes.bfloat16)                         # [128, G, K, HA]

    in_maps = []
    for c in range(N_CORES):
        r0 = c * R
        # adj_r[p, g, ib, i] = adj[r0 + ib*IW + i, g*128 + p]
        blk = adj[r0:r0 + R, :]                          # [R, N]
        adj_r = np.ascontiguousarray(
            blk.reshape(IB, IW, G, 128).transpose(3, 2, 0, 1)
        ).astype(ml_dtypes.bfloat16)                     # [128, G, IB, IW]
        # g1b[p, k, ib, i] = G1[r0 + ib*IW + i, k]
        g1_blk = G1[r0:r0 + R, :].reshape(IB, IW, K).transpose(2, 0, 1)
        g1b = np.broadcast_to(
            g1_blk[None].astype(ml_dtypes.bfloat16), (128, K, IB, IW))
        g1b = np.ascontiguousarray(g1b)
        in_maps.append({
            "adjr": adj_r,
            "g1b": g1b,
            "g2t": g2t,
            "hval": hval,
        })
    return in_maps


def kernel(features, adj, W, a):
    nc = build_program()
    in_maps = prepare_inputs(features, adj, W, a)
    res = run_bass_kernel_spmd(nc, in_maps, list(range(N_CORES)))
    out = np.concatenate(
        [res.results[c]["out"] for c in range(N_CORES)], axis=0)
    return out.astype(np.float32)


if __name__ == "__main__":
    rng = np.random.default_rng(0)
    features = rng.standard_normal((N, F_IN), dtype=np.float32)
    adj = (rng.integers(0, 2, size=(N, N))).astype(np.float32)
    W = (rng.standard_normal((K, F_IN, H), dtype=np.float32) * 0.118)
    a = (rng.standard_normal((K, 2 * H), dtype=np.float32) * 0.176)
    out = kernel(features=features, adj=adj, W=W, a=a)
    print("out", out.shape, out.dtype, np.abs(out).max())



# revision 20
# speedup vs baseline: 7.2678x; 1.7889x over previous
"""GAT layer (N=8192, F_IN=256, H=64 per head, K=8 heads) on 8 Trainium2 cores.

Strategy (row-sharding, fully data-parallel, no collectives):
  reference per head k:
    h   = features @ W[k]                      [N, H]
    wh1 = h @ a[k,:H]; wh2 = h @ a[k,H:]       [N]
    e   = leaky_relu(wh1[:,None] + wh2[None,:], 0.2)
    att = softmax(where(adj>0, e, -9e15), axis=1)
    out = elu(att @ h)

  Algebra: with s = wh1[i] + wh2[j],
    exp(lrelu(s)) = exp(0.2 s) * max(exp(0.8 s), 1)
                  = e1_{i} * e2_{j} * max(G1_i * G2_j, 1)
  where G1 = exp(0.8 wh1), G2 = exp(0.8 wh2), e1 = exp(0.2 wh1), e2 = exp(0.2 wh2).
  The row factor e1_i cancels in softmax.  The column factor e2_j is folded
  into the value matrix.  Masked entries: adj in {0,1} multiplies exactly.
    unnorm[i,j] = adj[i,j] * max(G1_i G2_j, 1) * e2_j   (up to the cancelled e1_i)
    out[i]      = elu( (unnorm @ h) / (unnorm @ 1) )
  On device the big [N/8, N] tensor per head is produced with only
  two DVE passes per tile (tensor_scalar mult+max fused, tensor_tensor mask)
  in bf16, and consumed by the PE with stationary [h*e2 | e2] per head.
  G1/G2/e2 come from tiny host matmuls (features @ (W[k] @ a[k])).

Per-core layout ([j,i]-transposed tiles so contraction j sits on partitions):
  adj_r  [128, 2, 64, 512] bf16 : adj_r[p,ib,g,i] = adj[r0+ib*512+i, g*128+p]
  g1b    [128, 8, 2, 512] bf16  : G1 row broadcast across partitions
  g2t    [128, 8, 64] f32       : g2t[p,k,g] = G2[g*128+p, k]  (per-partition scalars)
  e25t   [128, 64, 8] f32       : e25t[p,g,k] = e2[g*128+p, k]
  featT  [256, 8192] f32        : features.T (replicated; h computed on device)
  w_cat  [256, 512] f32         : all heads' W side by side
"""

import sys
import os

sys.path.insert(0, "/opt/trn_rl_repo")

import numpy as np
import ml_dtypes
from contextlib import ExitStack

import concourse.bass as bass
import concourse.tile as tile
from concourse import bacc, mybir
from concourse.bass_utils import run_bass_kernel_spmd

N = 8192
F_IN = 256
H = 64
K = 8
ALPHA = 0.2
N_CORES = 8
R = N // N_CORES          # 1024 rows per core
IB = 2                    # i-blocks per core (512 columns of out-rows each)
IW = R // IB              # 512, i-width per block
G = N // 128              # 64 j-groups of 128
G_SUB = 8                 # j-groups per adjacency DMA
HA = H + 1                # 65: head value columns + denominator column

F32 = mybir.dt.float32
BF16 = mybir.dt.bfloat16
AX = mybir.AluOpType

_cached = {}


def build_program(loop_t=1, z_eng="vvvvaaaa", m_eng="vvvvvvvv",
                  sweeps=((0, 1, 4, 5), (2, 3, 6, 7)), fold_e2=False):
    """z_eng[k]: engine producing head k's score tiles ('v'=DVE zm via
    tensor_scalar mult+max, 'a'=ACT v=relu(G2*g1b-1)=zm-1, 'g'=GPSIMD zm).
    m_eng[k]: engine for the mask multiply ('v' DVE / 'g' GPSIMD).
    ACT heads ('a') use [h*e2|e2] stationary and an extra correction
    matmul with rhs=adj; other heads fold e2 into the score via the
    tensor_scalar scalar2 AP: zm' = max(z*e2, e2).
    sweeps: two groups of 4 heads; each sweep holds 8 PSUM accumulators
    ([65,512] x 2 i-blocks) and re-reads the adjacency."""
    key = (loop_t, z_eng, m_eng, tuple(map(tuple, sweeps)), fold_e2)
    if key in _cached:
        return _cached[key]

    nc = bacc.Bacc("TRN2", target_bir_lowering=False, debug=False,
                   num_devices=N_CORES)

    adjr_d = nc.dram_tensor("adjr", [128, G, IB, IW], BF16, kind="ExternalInput").ap()
    g1b_d = nc.dram_tensor("g1b", [128, K, IB, IW], BF16, kind="ExternalInput").ap()
    g2t_d = nc.dram_tensor("g2t", [128, K, G], F32, kind="ExternalInput").ap()
    hval_d = nc.dram_tensor("hval", [128, G, K, HA], BF16,
                            kind="ExternalInput").ap()
    out_d = nc.dram_tensor("out", [R, K * H], F32, kind="ExternalOutput").ap()

    with tile.TileContext(nc) as tc:
        with ExitStack() as ctx:
            const = ctx.enter_context(tc.tile_pool(name="const", bufs=1))
            hpool = ctx.enter_context(tc.tile_pool(name="h", bufs=G))
            fpool = ctx.enter_context(tc.tile_pool(name="feat", bufs=4))
            e25x_pool = ctx.enter_context(tc.tile_pool(name="e25x", bufs=2))
            adj_pool = ctx.enter_context(tc.tile_pool(name="adj", bufs=2))
            z_pool = ctx.enter_context(tc.tile_pool(name="z", bufs=3))
            u_pool = ctx.enter_context(tc.tile_pool(name="u", bufs=3))
            stg_pool = ctx.enter_context(tc.tile_pool(name="stg", bufs=4))
            fin_pool = ctx.enter_context(tc.tile_pool(name="fin", bufs=2))
            acc_sb_pool = ctx.enter_context(tc.tile_pool(name="accsb", bufs=2))
            psum = ctx.enter_context(tc.tile_pool(name="psum", bufs=8, space="PSUM"))

            # ---- constants ----
            g1b_sb = const.tile([128, K, IB, IW], BF16)
            nc.sync.dma_start(g1b_sb[:], g1b_d[:])
            g2t_sb = const.tile([128, K, G], F32)
            nc.sync.dma_start(g2t_sb[:], g2t_d[:])
            # value matrix [h*e2 | e2] per (g, k): computed on host now
            hval_sb = const.tile([128, G, K, HA], BF16)
            nc.sync.dma_start(hval_sb[:], hval_d[:])
            ident = const.tile([128, 128], F32)
            from concourse.masks import make_identity
            make_identity(nc, ident[:])
            negone = const.tile([128, 1], F32)
            nc.vector.memset(negone[:], -1.0)

            loop_cm = tc.For_i(0, loop_t, 1) if loop_t > 1 else None
            if loop_cm is not None:
                ctx.enter_context(loop_cm)

            assert not fold_e2, "host hval provides [h*e2|e2] (act-type) only"
            act_heads = list(range(K))
            h_sb = [hval_sb[:, g, :, :] for g in range(G)]

            # ---- phase B: two 4-head sweeps over the adjacency ----
            assert fold_e2 or len(act_heads) == K
            PAIR = IB * IW  # 1024

            for sweep_heads in sweeps:
                accs = {}
                for k in sweep_heads:
                    for ib in range(IB):
                        accs[(k, ib)] = psum.tile(
                            [HA, IW], F32, tag="ps", name=f"acc{k}_{ib}")
                for gs in range(G // G_SUB):
                    adj_t = adj_pool.tile([128, G_SUB, IB, IW], BF16)
                    nc.sync.dma_start(
                        adj_t[:], adjr_d[:, gs * G_SUB:(gs + 1) * G_SUB, :, :]
                    )
                    for gi in range(G_SUB):
                        g = gs * G_SUB + gi
                        adj_pair = adj_t[:, gi, :, :].rearrange("p b i -> p (b i)")
                        z_all = z_pool.tile([128, 4, PAIR], BF16, tag="z")
                        for s, k in enumerate(sweep_heads):
                            g1b_pair = g1b_sb[:, k, :, :].rearrange(
                                "p b i -> p (b i)")
                            s1 = g2e_sb[:, k, g:g + 1] if fold_e2 else \
                                g2t_sb[:, k, g:g + 1]
                            s2 = e25t_sb[:, g, k:k + 1] if fold_e2 else 1.0
                            if z_eng[k] == "v":
                                # zm' = max(z*e2, e2) -- e2 folded in
                                nc.vector.tensor_scalar(
                                    z_all[:, s, :], g1b_pair, s1, s2,
                                    op0=AX.mult, op1=AX.max,
                                )
                            elif z_eng[k] == "g":
                                nc.gpsimd.tensor_scalar(
                                    z_all[:, s, :], g1b_pair, s1, s2,
                                    op0=AX.mult, op1=AX.max,
                                )
                            else:
                                # v = relu(G2*g1b - 1) = zm - 1 (unscaled)
                                nc.scalar.activation(
                                    z_all[:, s, :], g1b_pair,
                                    mybir.ActivationFunctionType.Relu,
                                    bias=negone[:],
                                    scale=g2t_sb[:, k, g:g + 1],
                                )
                        u_all = u_pool.tile([128, 4, PAIR], BF16, tag="u")
                        for s, k in enumerate(sweep_heads):
                            eng = nc.gpsimd if m_eng[k] == "g" else nc.vector
                            eng.tensor_tensor(
                                u_all[:, s, :], z_all[:, s, :], adj_pair,
                                op=AX.mult)
                        for s, k in enumerate(sweep_heads):
                            is_act = z_eng[k] == "a"
                            for ib in range(IB):
                                nc.tensor.matmul(
                                    accs[(k, ib)][:],
                                    h_sb[g][:, k, :],
                                    u_all[:, s, ib * IW:(ib + 1) * IW],
                                    start=(g == 0),
                                    stop=(g == G - 1 and not is_act),
                                )
                            if is_act:  # correction: acc += h_sb @ adj
                                for ib in range(IB):
                                    nc.tensor.matmul(
                                        accs[(k, ib)][:],
                                        h_sb[g][:, k, :],
                                        adj_t[:, gi, ib, :],
                                        start=False,
                                        stop=(g == G - 1),
                                    )
                # drain this sweep: transpose, divide, elu, store
                sw_sorted = sorted(sweep_heads)
                # contiguous head runs for the output DMA
                runs = []
                for k in sw_sorted:
                    if runs and runs[-1][-1] == k - 1:
                        runs[-1].append(k)
                    else:
                        runs.append([k])
                for ib in range(IB):
                    stgs = [stg_pool.tile([128, 4, HA], F32, tag="stg",
                                          name=f"stg{sweep_heads[0]}_{ib}_{c}")
                            for c in range(IW // 128)]
                    for s, k in enumerate(sw_sorted):
                        acc_sb = acc_sb_pool.tile([HA, IW], F32, tag="accsb")
                        nc.scalar.copy(acc_sb[:], accs[(k, ib)][:])
                        for c in range(IW // 128):
                            pst = psum.tile([128, HA], F32, tag="ps",
                                            name=f"pst{k}_{ib}_{c}")
                            nc.tensor.transpose(
                                pst[:], acc_sb[:, c * 128:(c + 1) * 128],
                                ident[0:HA, 0:HA],
                            )
                            nc.scalar.copy(stgs[c][:, s, :], pst[:])
                    for c in range(IW // 128):
                        stg = stgs[c]
                        recips = fin_pool.tile([128, 4], F32, tag="recip")
                        nc.vector.reciprocal(recips[:], stg[:, :, H])
                        fin = fin_pool.tile([128, 4, H], F32, tag="fin")
                        nc.vector.tensor_tensor(
                            fin[:], stg[:, :, 0:H],
                            recips[:].unsqueeze(2).broadcast_to((128, 4, H)),
                            op=AX.mult,
                        )
                        # elu(x) = exp(min(x,0)) + (max(x,0) - 1)
                        fin2 = fin_pool.tile([128, 4 * H], F32, tag="fin2")
                        finf = fin[:].rearrange("p k f -> p (k f)")
                        nc.vector.tensor_scalar(
                            fin2[:], finf, 0.0, None, op0=AX.min
                        )
                        ex = fin_pool.tile([128, 4 * H], F32, tag="ex")
                        nc.scalar.activation(
                            ex[:], fin2[:], mybir.ActivationFunctionType.Exp
                        )
                        rel = fin_pool.tile([128, 4 * H], F32, tag="rel")
                        nc.vector.tensor_scalar(
                            rel[:], finf, 0.0, -1.0, op0=AX.max, op1=AX.add
                        )
                        res = fin_pool.tile([128, 4 * H], F32, tag="res")
                        nc.vector.tensor_tensor(res[:], ex[:], rel[:], op=AX.add)
                        resv = res[:].rearrange("p (k f) -> p k f", k=4)
                        for run in runs:
                            s0 = sw_sorted.index(run[0])
                            nc.sync.dma_start(
                                out_d[ib * IW + c * 128:
                                      ib * IW + (c + 1) * 128,
                                      run[0] * H:(run[-1] + 1) * H],
                                resv[:, s0:s0 + len(run), :],
                            )

    nc.compile()
    _cached[key] = nc
    return nc


def prepare_inputs(features, adj, W, a):
    """Host-side prep: tiny projections + per-core sharded/transposed layouts."""
    features = np.asarray(features, dtype=np.float32)
    adj = np.asarray(adj, dtype=np.float32)
    W = np.asarray(W, dtype=np.float32)
    a = np.asarray(a, dtype=np.float32)

    # av[k] = W[k] @ a[k]  -> wh = features @ av.T   (tiny: K*F_IN*H flops)
    av1 = np.einsum("kfh,kh->kf", W, a[:, :H])          # [K, F_IN]
    av2 = np.einsum("kfh,kh->kf", W, a[:, H:])          # [K, F_IN]
    wh1 = features @ av1.T                               # [N, K]
    wh2 = features @ av2.T                               # [N, K]
    G1 = np.exp(0.8 * wh1).astype(np.float32)            # row factors
    G2 = np.exp(0.8 * wh2).astype(np.float32)            # col factors
    E25 = np.exp(0.2 * wh2).astype(np.float32)           # folded into values

    # g2t[p,k,g] = G2[g*128+p, k]
    g2t = np.ascontiguousarray(
        G2.reshape(G, 128, K).transpose(1, 2, 0))        # [128, K, G]

    # host value matrix: hval[p, g, k, :] = [h*e2 | e2] at node g*128+p
    h = np.einsum("nf,kfh->nkh", features, W, optimize=True)  # [N, K, H]
    hval = np.empty((N, K, HA), dtype=np.float32)
    hval[:, :, 0:H] = h * E25[:, :, None]
    hval[:, :, H] = E25
    hval = np.ascontiguousarray(
        hval.reshape(G, 128, K, HA).transpose(1, 0, 2, 3)
    ).astype(ml_dtypes.bfloat16)                         # [128, G, K, HA]

    in_maps = []
    for c in range(N_CORES):
        r0 = c * R
        # adj_r[p, g, ib, i] = adj[r0 + ib*IW + i, g*128 + p]
        blk = adj[r0:r0 + R, :]                          # [R, N]
        adj_r = np.ascontiguousarray(
            blk.reshape(IB, IW, G, 128).transpose(3, 2, 0, 1)
        ).astype(ml_dtypes.bfloat16)                     # [128, G, IB, IW]
        # g1b[p, k, ib, i] = G1[r0 + ib*IW + i, k]
        g1_blk = G1[r0:r0 + R, :].reshape(IB, IW, K).transpose(2, 0, 1)
        g1b = np.broadcast_to(
            g1_blk[None].astype(ml_dtypes.bfloat16), (128, K, IB, IW))
        g1b = np.ascontiguousarray(g1b)
        in_maps.append({
            "adjr": adj_r,
            "g1b": g1b,
            "g2t": g2t,
            "hval": hval,
        })
    return in_maps


def kernel(features, adj, W, a):
    nc = build_program()
    in_maps = prepare_inputs(features, adj, W, a)
    res = run_bass_kernel_spmd(nc, in_maps, list(range(N_CORES)))
    out = np.concatenate(
        [res.results[c]["out"] for c in range(N_CORES)], axis=0)
    return out.astype(np.float32)


if __name__ == "__main__":
    rng = np.random.default_rng(0)
    features = rng.standard_normal((N, F_IN), dtype=np.float32)
    adj = (rng.integers(0, 2, size=(N, N))).astype(np.float32)
    W = (rng.standard_normal((K, F_IN, H), dtype=np.float32) * 0.118)
    a = (rng.standard_normal((K, 2 * H), dtype=np.float32) * 0.176)
    out = kernel(features=features, adj=adj, W=W, a=a)
    print("out", out.shape, out.dtype, np.abs(out).max())

